# revision 37
# baseline (speedup 1.0000x reference)
"""Trainium2 Bass kernel for nn_Block_13709535609166 (dense transformer block).

B=8, T=1024, D=1024, H=16, HD=64, FF=4096. Data-parallel over batch: one
batch element per NeuronCore (8 cores), no collectives. All matmuls bf16
with fp32 PSUM accumulation; residual stream and LN/softmax arithmetic fp32.

Self-contained: hardcodes shapes/sharding; only needs numpy/ml_dtypes and
the concourse (Bass) stack available in the container image.
"""
import numpy as np
import ml_dtypes

import concourse.bass as bass
import concourse.mybir as mybir
import concourse.tile as tile
from concourse import bacc
from concourse.masks import make_identity

BF16 = mybir.dt.bfloat16
F8 = mybir.dt.float8e4
F32 = mybir.dt.float32
AF = mybir.ActivationFunctionType
ALU = mybir.AluOpType
DR = mybir.MatmulPerfMode.DoubleRow
GELU_AF = AF.Gelu  # swapped to Tanh for CoreSim (Gelu not implemented in sim)

B, T, D, H = 8, 1024, 1024, 16
HD = D // H  # 64
FF = 4 * D
TC = T // 128   # 8 token chunks
DC = D // 128   # 8 feature chunks
DC2 = DC // 2   # 4 fp8 DoubleRow pair chunks
FC = FF // 128  # 32 ff chunks
NT = T // 512   # 2 free-dim chunks of 512 tokens
NF = D // 512   # 2 free-dim chunks of 512 features
W8SCALE = 64.0  # fp8 weights are stored x64 to stay out of subnormals


def build_block_kernel(nc, reps=1, loop_n=0):
    """Emit the full transformer block for one batch element.

    reps>1 re-applies the block on its own output (SBUF-chained); loop_n>0
    wraps the body in a hardware For loop -- both only for timing NEFFs so
    the (tens of ms) axon RPC overhead can be divided away.
    """
    dram = {}
    for name, shape, dt in [
        ("x", [T, D], F32),
        ("wq", [D, D], F8), ("wk", [D, D], F8), ("wv", [D, D], F8),
        ("bq", [D], F32), ("bk", [D], F32), ("bv", [D], F32),
        ("wo", [D, D], F8), ("bo", [D], F32), ("bo_bf", [D], BF16), ("b2_bf", [D], BF16), ("bv_bf", [D], BF16),
        ("w1", [FC, 128, DC, 128], BF16), ("b1", [FF], F32),
        ("w2", [FC, NF, 128, 512], BF16), ("b2", [D], F32),
        ("ln1_g", [D], F32), ("ln1_b", [D], F32),
        ("ln2_g", [D], F32), ("ln2_b", [D], F32),
    ]:
        dram[name] = nc.dram_tensor(name, shape, dt, kind="ExternalInput").ap()
    out_d = nc.dram_tensor("out", [T, D], F32, kind="ExternalOutput").ap()
    out_r = out_d.rearrange("(m p) d -> p m d", p=128)

    with tile.TileContext(nc) as tc:
        _emit(nc, tc, dram, out_r, reps, loop_n)
    return nc


def _emit(nc, tc, dram, out_r, reps=1, loop_n=0):
    from contextlib import ExitStack

    with ExitStack() as ctx:
        consts = ctx.enter_context(tc.tile_pool(name="consts", bufs=1))
        resid = ctx.enter_context(tc.tile_pool(name="resid", bufs=1))
        work = ctx.enter_context(tc.tile_pool(name="work", bufs=4))

        # ---- constants ----
        ident = consts.tile([128, 128], BF16)
        make_identity(nc, ident)
        ones65 = consts.tile([65, 64], BF16)
        nc.vector.memset(ones65, 1.0)
        eps_t = consts.tile([128, 1], F32)
        nc.vector.memset(eps_t, 1e-5)

        # ---- residual stream (token-major fp32, updated in place) ----
        x_sb = resid.tile([128, TC, D], F32)
        x_r = dram["x"].rearrange("(m p) d -> p m d", p=128)
        nc.sync.dma_start(x_sb[:, 0, :], x_r[:, 0, :])
        nc.sync.dma_start(x_sb[:, 1, :], x_r[:, 1, :])

        col = {}
        for name in ["bq", "bk"]:
            col[name] = consts.tile([128, DC], F32, name=f"c_{name}")
            nc.gpsimd.dma_start(col[name], dram[name].rearrange("(o p) -> p o", p=128))
        col["b1"] = consts.tile([128, FC], F32, name="c_b1")
        nc.gpsimd.dma_start(col["b1"], dram["b1"].rearrange("(o p) -> p o", p=128))
        # bv/bo/b2 are folded into the matmul accumulation via a ones-row matmul
        ones_row = consts.tile([1, 128], BF16, name="ones_row")
        nc.vector.memset(ones_row, 1.0)
        brow = {}
        for name in ["bv", "bo", "b2"]:
            brow[name] = consts.tile([1, D], BF16, name=f"br_{name}")
            nc.gpsimd.dma_start(brow[name], dram[name + "_bf"][None, :])

        hnorm_pool = ctx.enter_context(tc.tile_pool(name="hnorm", bufs=4))

        def ln_stats(x_src, m):
            """Token-major LN stats for chunk m -> normalized bf16 tile."""
            stats = work.tile([128, 2, 6], F32, name="stats")
            nc.vector.bn_stats(out=stats[:, 0, :], in_=x_src[:, m, 0:512])
            nc.vector.bn_stats(out=stats[:, 1, :], in_=x_src[:, m, 512:1024])
            mv = work.tile([128, 2], F32, name="mv")
            nc.vector.bn_aggr(out=mv, in_=stats)
            # rstd = exp(-0.5*ln(var+eps)). Ln+Exp share one ACT table set
            # (natural_log_exp_and_others) with the softmax Exp, so LN2 stats
            # in the attention window never force a ~2.7us table reload the
            # way Sqrt (its own set) would.
            lnv = work.tile([128, 1], F32, name="lnv")
            nc.scalar.activation(out=lnv, in_=mv[:, 1:2], func=AF.Ln,
                                 bias=eps_t, scale=1.0)
            rstd = work.tile([128, 1], F32, name="rstd")
            nc.scalar.activation(out=rstd, in_=lnv, func=AF.Exp, scale=-0.5)
            h_norm = hnorm_pool.tile([128, D], BF16, name="h_norm")
            nc.any.tensor_scalar(
                out=h_norm, in0=x_src[:, m, :], scalar1=mv[:, 0:1], scalar2=rstd,
                op0=ALU.subtract, op1=ALU.mult)
            return h_norm

        def ln_transpose(h_norm, m, h_fm, tr_pool, tag="tr"):
            # LN gain/bias are folded into the weights/biases on the host, so
            # the transpose drain is a plain copy; nc.any lets the scheduler
            # put it on whichever of ACT/DVE is idle in this phase.
            for ko in range(DC):
                tr_ps = tr_pool.tile([128, 128], BF16, name=tag)
                nc.tensor.transpose(tr_ps, h_norm[:, 128 * ko:128 * ko + 128], ident)
                nc.any.tensor_copy(
                    out=h_fm[:, ko, 128 * m:128 * m + 128], in_=tr_ps)

        from contextlib import nullcontext
        loop_ctx = tc.For_i(0, loop_n, 1) if loop_n else nullcontext()
        with loop_ctx:
         for rep in range(reps):
            # Long-lived pools, opened early to respect LIFO pool discipline.
            # Stack (bottom->top): h2 | ctx | wo | qkv | <phase-local pools>
            st = ExitStack()
            p_h2 = st.enter_context(tc.tile_pool(name="h2_sb", bufs=1))
            h2_bf = p_h2.tile([128, DC, T], BF16, name="h2_bf")
            st_ctx = ExitStack()
            p_ctx = st_ctx.enter_context(tc.tile_pool(name="ctx_sb", bufs=1))
            ctx8 = p_ctx.tile([128, DC2, 2, T], F8, name="ctx8")
            st_wo = ExitStack()
            p_wo = st_wo.enter_context(tc.tile_pool(name="wo_sb", bufs=1))
            wo_sb = p_wo.tile([128, DC2, 2, D], F8, name="wo_sb")
            st_qkv = ExitStack()
            p_qkv = st_qkv.enter_context(tc.tile_pool(name="qkv_sb", bufs=1))
            q_fm = p_qkv.tile([128, DC, T], BF16, name="q_fm")
            k_fm = p_qkv.tile([128, DC, T], BF16, name="k_fm")
            # V with a ones column appended per head: [s, head, 0:64]=V, [.,.,64]=1
            # so the ctx matmul also produces the softmax denominator in row 64.
            v65 = p_qkv.tile([128, TC, H, 65], BF16, name="v65")

            # ================= Phase 1: LN1 + QKV (fp8 DoubleRow) ==========
            # Weights live x64-scaled in fp8; h (LN1 out) is cast to fp8 in
            # the transpose-apply. ko chunk pairs are contracted 2-at-a-time
            # via perf_mode=DoubleRow (lhsT AP [128, 2, m], rhs [128, 2, n]).
            st_p1 = ExitStack()
            p1 = st_p1.enter_context(tc.tile_pool(name="p1_sb", bufs=1))
            tr_pool = st_p1.enter_context(tc.tile_pool(name="p1_tr", bufs=3, space="PSUM"))
            mm_pool = st_p1.enter_context(tc.tile_pool(name="p1_mm", bufs=5, space="PSUM"))

            h8 = p1.tile([128, DC2, 2, T], F8, name="h8")
            h8_flat = h8.rearrange("p a b t -> p (a b) t")
            h_norms = [None] * TC
            h_norms[0] = ln_stats(x_sb, 0)
            h_norms[1] = ln_stats(x_sb, 1)
            w8r = {n: dram[n].rearrange("(pr j p) q -> p pr j q", p=128, j=2)
                   for n in ("wq", "wk", "wv", "wo")}

            with tc.tile_pool(name="p1_w", bufs=3) as p_w:
                wv_sb = p_w.tile([128, DC2, 2, D], F8, name="w")
                wk_sb = p_w.tile([128, DC2, 2, D], F8, name="w")
                wq_sb = p_w.tile([128, DC2, 2, D], F8, name="w")
                for _pr in range(DC2):
                    for _j in range(2):
                        nc.sync.dma_start(wv_sb[:, _pr, _j, :], w8r["wv"][:, _pr, _j, :])
                    nc.sync.dma_start(x_sb[:, _pr + 2, :], x_r[:, _pr + 2, :])
                for _pr in range(DC2):
                    for _j in range(2):
                        nc.sync.dma_start(wk_sb[:, _pr, _j, :], w8r["wk"][:, _pr, _j, :])
                    if _pr < 2:
                        nc.sync.dma_start(x_sb[:, _pr + 6, :], x_r[:, _pr + 6, :])
                for _pr in range(DC2):
                    for _j in range(2):
                        nc.sync.dma_start(wq_sb[:, _pr, _j, :], w8r["wq"][:, _pr, _j, :])
                nc.vector.memset(v65[:, :, :, 64:65], 1.0)
                # V: token-major [t, vfeat]; lhsT = h8 pair chunk, rhs = W.
                # Transposes for chunk m emitted right before V[m] so PE never
                # waits in-order on DVE-produced h_norm.
                for m in range(TC):
                    ln_transpose(h_norms[m], m, h8_flat, tr_pool)
                    if m + 2 < TC:
                        h_norms[m + 2] = ln_stats(x_sb, m + 2)
                    ps = [mm_pool.tile([128, 512], F32, name="mm") for _ in range(NF)]
                    for pr in range(DC2):
                        for fn in range(NF):
                            nc.tensor.matmul(
                                ps[fn], lhsT=h8[:, pr, :, 128 * m:128 * m + 128],
                                rhs=wv_sb[:, pr, :, 512 * fn:512 * fn + 512],
                                start=(pr == 0), stop=False, perf_mode=DR)
                    for fn in range(NF):
                        nc.tensor.matmul(ps[fn], lhsT=ones_row,
                                         rhs=brow["bv"][:, 512 * fn:512 * fn + 512],
                                         start=False, stop=True)
                        nc.scalar.activation(
                            out=v65[:, m, 8 * fn:8 * fn + 8, 0:64],
                            in_=ps[fn].rearrange("p (h e) -> p h e", e=64),
                            func=AF.Copy, scale=1.0 / W8SCALE)
                # K, Q interleaved per m so attention pair m can start early
                for m in range(DC):
                    for dst, wt, bias in [(k_fm, wk_sb, col["bk"]), (q_fm, wq_sb, col["bq"])]:
                        ps = [mm_pool.tile([128, 512], F32, name="mm") for _ in range(NT)]
                        for pr in range(DC2):
                            for tn in range(NT):
                                nc.tensor.matmul(
                                    ps[tn], lhsT=wt[:, pr, :, 128 * m:128 * m + 128],
                                    rhs=h8[:, pr, :, 512 * tn:512 * tn + 512],
                                    start=(pr == 0), stop=(pr == DC2 - 1), perf_mode=DR)
                        for tn in range(NT):
                            nc.any.tensor_scalar(
                                out=dst[:, m, 512 * tn:512 * tn + 512], in0=ps[tn],
                                scalar1=1.0 / W8SCALE, scalar2=bias[:, m:m + 1],
                                op0=ALU.mult, op1=ALU.add)
            st_p1.close()

            # ========== Phase 2+3: attention / out-proj pipelined ==========
            for _pr in range(DC2):
                for _j in range(2):
                    nc.sync.dma_start(wo_sb[:, _pr, _j, :], w8r["wo"][:, _pr, _j, :])

            h2_norms = [None] * TC
            MH = TC // 2  # token chunks per tn half

            def proj_m(m, mm3_pool):
                """Out-proj + residual + LN2 + h2 transpose for token chunk m."""
                for fn in range(NF):
                    sl = slice(512 * fn, 512 * fn + 512)
                    ps = mm3_pool.tile([128, 512], F32, name="pr")
                    for pr in range(DC2):
                        nc.tensor.matmul(
                            ps, lhsT=ctx8[:, pr, :, 128 * m:128 * m + 128],
                            rhs=wo_sb[:, pr, :, sl],
                            start=(pr == 0), stop=False, perf_mode=DR)
                    nc.tensor.matmul(ps, lhsT=ones_row, rhs=brow["bo"][:, sl],
                                     start=False, stop=True)
                    # x += attn_out/W8SCALE  (bias row is pre-scaled on host)
                    nc.vector.scalar_tensor_tensor(
                        out=x_sb[:, m, sl], in0=ps, scalar=1.0 / W8SCALE,
                        in1=x_sb[:, m, sl], op0=ALU.mult, op1=ALU.add)
                h2_norms[m] = ln_stats(x_sb, m)
                ln_transpose(h2_norms[m], m, h2_bf, mm3_pool, tag="pr")

            # PSUM stack: pr (bottom, 2 banks) | sc (4 banks) | cx (2 banks)
            st_pr = ExitStack()
            pr_pool = st_pr.enter_context(tc.tile_pool(name="p3_pr", bufs=2, space="PSUM"))
            st_att = ExitStack()
            e_pool = st_att.enter_context(tc.tile_pool(name="p2_e", bufs=6))
            sc_pool = st_att.enter_context(tc.tile_pool(name="p2_sc", bufs=2, space="PSUM"))
            cx_pool = st_att.enter_context(tc.tile_pool(name="p2_cx", bufs=1, space="PSUM"))
            # Flat software pipeline over (tn, pc, sm): the two heads' score
            # MMs are adjacent (disjoint PE row groups 0-63/64-127 -> they
            # run concurrently in the array on HW); ctx MMs trail the exp by
            # DEPTH steps; each head-pair's normalize chain is deferred into
            # the next pair's stream so neither PE nor ACT ever waits on it.
            DEPTH = 3
            steps = [(tn, pc, sm)
                     for tn in range(NT) for pc in range(DC) for sm in range(TC)]
            cx_of = {}   # (tn, pc) -> [cx_ps_h0, cx_ps_h1]
            e_of = {}    # step idx -> e tile

            def ctx_mms(idx):
                tn, pc, sm = steps[idx]
                if sm == 0:
                    cx_of[(tn, pc)] = [cx_pool.tile([65, 512], F32, name=f"cx{h}")
                                       for h in range(2)]
                e = e_of.pop(idx)
                for h in range(2):
                    nc.tensor.matmul(
                        cx_of[(tn, pc)][h], lhsT=v65[:, sm, 2 * pc + h, :],
                        rhs=e[:, h, :],
                        start=(sm == 0), stop=(sm == TC - 1))

            def normalize(tn, pc):
                tsl = slice(512 * tn, 512 * tn + 512)
                cx_ps = cx_of.pop((tn, pc))
                for h in range(2):
                    lo = 64 * h
                    rec = work.tile([65, 512], BF16, name="rec")
                    with nc.allow_low_precision("softmax denom recip in bf16"):
                        nc.vector.reciprocal(rec[64:65, :], cx_ps[h][64:65, :])
                    rb_ps = pr_pool.tile([128, 512], F32, name="pr")
                    nc.tensor.matmul(
                        rb_ps[0:64, :], lhsT=ones65[64:65, :], rhs=rec[64:65, :],
                        start=True, stop=True)
                    # walrus forbids two PSUM reads in one DVE op -> copy the
                    # unnormalized ctx (can be ~1e3, too big for fp8) to a
                    # bf16 staging tile, then scale into the fp8 ctx store.
                    tmp = work.tile([64, 512], BF16, name="cxc")
                    nc.vector.tensor_copy(out=tmp, in_=cx_ps[h][0:64, :])
                    nc.vector.tensor_mul(out=ctx8[lo:lo + 64, pc // 2, pc % 2, tsl],
                                         in0=tmp, in1=rb_ps[0:64, :])

            for idx, (tn, pc, sm) in enumerate(steps):
                tsl = slice(512 * tn, 512 * tn + 512)
                sc = sc_pool.tile([128, 2, 512], F32, name="sc")
                for h in range(2):
                    lo = 64 * h
                    nc.tensor.matmul(
                        sc[:, h, :],
                        lhsT=k_fm[lo:lo + 64, pc, 128 * sm:128 * sm + 128],
                        rhs=q_fm[lo:lo + 64, pc, tsl],
                        start=True, stop=True)
                e = e_of[idx] = e_pool.tile([128, 2, 512], BF16, name="e")
                nc.scalar.activation(out=e, in_=sc, func=AF.Exp, scale=0.125)
                if idx >= DEPTH:
                    ctx_mms(idx - DEPTH)
                if sm == DEPTH - 1 and idx >= TC:
                    ptn, ppc, _ = steps[idx - TC]
                    normalize(ptn, ppc)
                # proj of the first token half interleaves into the second
                # attention half's emission so pr-slot allocation follows
                # usage order (no blocking behind later rb tiles).
                if tn == 1 and sm == 6 and pc % 2 == 1:
                    proj_m((pc - 1) // 2, pr_pool)
            for idx in range(len(steps) - DEPTH, len(steps)):
                ctx_mms(idx)
            normalize(*steps[-1][:2])
            for m in range(MH, TC):
                proj_m(m, pr_pool)
            st_att.close()
            st_qkv.close()

            # ================= Phase 4: FFN =================
            st_g1 = ExitStack()
            p_g1 = st_g1.enter_context(tc.tile_pool(name="g1_sb", bufs=1))
            g1_fm = p_g1.tile([128, FC, T], BF16, name="g1_fm")

            st_mm4 = ExitStack()
            mm4_pool = st_mm4.enter_context(
                tc.tile_pool(name="p4_mm1", bufs=4, space="PSUM"))
            st_w1 = ExitStack()
            w1_pool = st_w1.enter_context(tc.tile_pool(name="p4_w1", bufs=6))
            # mm1 in two passes (tn halves) so the tn=0 pass overlaps the tail
            # of attention/proj; W1 is re-streamed for the second pass.
            for tn in range(NT):
                tsl = slice(512 * tn, 512 * tn + 512)
                for mf in range(FC):
                    w1_t = w1_pool.tile([128, DC, 128], BF16, name="w1t")
                    nc.sync.dma_start(w1_t, dram["w1"][mf])
                    ps = mm4_pool.tile([128, 512], F32, name="mm")
                    for ko in range(DC):
                        nc.tensor.matmul(
                            ps, lhsT=w1_t[:, ko, :], rhs=h2_bf[:, ko, tsl],
                            start=(ko == 0), stop=(ko == DC - 1))
                    nc.scalar.activation(
                        out=g1_fm[:, mf, tsl], in_=ps,
                        func=GELU_AF, bias=col["b1"][:, mf:mf + 1], scale=1.0)
            st_w1.close()
            st_mm4.close()
            st_pr.close()

            st_mm4b = ExitStack()
            mm4b_pool = st_mm4b.enter_context(
                tc.tile_pool(name="p4_mm2", bufs=8, space="PSUM"))
            st_w2 = ExitStack()
            w2_pool = st_w2.enter_context(tc.tile_pool(name="p4_w2", bufs=4))
            with (
                tc.tile_pool(name="p4_out", bufs=4) as out_pool,
            ):
                for th in range(2):
                    ps = [[mm4b_pool.tile([128, 512], F32, name="mm") for _ in range(NF)]
                          for _ in range(MH)]
                    for ko in range(FC):
                        w2_t = w2_pool.tile([128, NF, 512], BF16, name="w2t")
                        nc.sync.dma_start(w2_t, dram["w2"][ko].rearrange("f p c -> p f c"))
                        for m4 in range(MH):
                            m = MH * th + m4
                            for fn in range(NF):
                                nc.tensor.matmul(
                                    ps[m4][fn], lhsT=g1_fm[:, ko, 128 * m:128 * m + 128],
                                    rhs=w2_t[:, fn, :],
                                    start=(ko == 0), stop=False)
                    for m4 in range(MH):
                        m = MH * th + m4
                        for fn in range(NF):
                            sl = slice(512 * fn, 512 * fn + 512)
                            nc.tensor.matmul(ps[m4][fn], lhsT=ones_row,
                                             rhs=brow["b2"][:, sl], start=False, stop=True)
                            o = out_pool.tile([128, 512], F32, name="o")
                            nc.vector.tensor_add(out=o, in0=ps[m4][fn], in1=x_sb[:, m, sl])
                            if rep < reps - 1 or loop_n:
                                nc.vector.tensor_scalar(
                                    out=x_sb[:, m, sl], in0=o, scalar1=0.5,
                                    scalar2=None, op0=ALU.mult)
                            if rep == reps - 1:
                                nc.sync.dma_start(out_r[:, m, sl], o)
            st_w2.close()
            st_mm4b.close()
            st_g1.close()
            st_wo.close()
            st_ctx.close()
            st.close()


_BUILT = {}


def _get_built():
    if "nc" not in _BUILT:
        nc = bacc.Bacc("TRN2", target_bir_lowering=False, debug=False,
                       enable_asserts=False, num_devices=8)
        build_block_kernel(nc)
        nc.compile()
        _BUILT["nc"] = nc
    return _BUILT["nc"]


def prep_inputs(inputs):
    """Host-side reshape/cast of the full (unsharded) inputs.

    LayerNorm affine params are folded in here: gains scale the weight rows
    (h_norm @ diag(g) @ W == h_norm @ (g[:,None]*W)) and biases fold into the
    downstream bias vectors (b_ln @ W + b).
    """
    bf = ml_dtypes.bfloat16
    f8 = ml_dtypes.float8_e4m3
    f32 = np.float32

    g1 = np.asarray(inputs["ln1_g"], f32)
    b1ln = np.asarray(inputs["ln1_b"], f32)
    g2 = np.asarray(inputs["ln2_g"], f32)
    b2ln = np.asarray(inputs["ln2_b"], f32)

    def flat_heads(w):  # [H, D, HD] -> [D, H*HD] f32
        return np.ascontiguousarray(np.transpose(np.asarray(w, f32), (1, 0, 2))
                                    .reshape(D, D))

    Wq_f, Wk_f, Wv_f = (flat_heads(inputs[n]) for n in ("Wq", "Wk", "Wv"))
    bq_e = np.asarray(inputs["bq"], f32).reshape(D) + b1ln @ Wq_f
    bk_e = np.asarray(inputs["bk"], f32).reshape(D) + b1ln @ Wk_f
    bv_e = np.asarray(inputs["bv"], f32).reshape(D) + b1ln @ Wv_f
    W1_f = np.asarray(inputs["W1"], f32)
    b1_e = np.asarray(inputs["b1"], f32) + b2ln @ W1_f

    common = {
        "wq": (g1[:, None] * Wq_f * W8SCALE).astype(f8),
        "wk": (g1[:, None] * Wk_f * W8SCALE).astype(f8),
        "wv": (g1[:, None] * Wv_f * W8SCALE).astype(f8),
        "bq": bq_e, "bk": bk_e, "bv": bv_e,
        "wo": (np.ascontiguousarray(np.asarray(inputs["Wo"], f32)) * W8SCALE).astype(f8),
        "bo": np.asarray(inputs["bo"], f32).copy(),
        "bo_bf": (np.asarray(inputs["bo"], f32) * W8SCALE).astype(bf),
        "b2_bf": np.asarray(inputs["b2"], f32).astype(bf),
        "bv_bf": (bv_e * W8SCALE).astype(bf),
        "w1": np.ascontiguousarray(
            (g2[:, None] * W1_f).reshape(DC, 128, FC, 128)
            .transpose(2, 1, 0, 3)).astype(bf),
        "b1": b1_e,
        "w2": np.ascontiguousarray(
            np.asarray(inputs["W2"], f32).reshape(FC, 128, NF, 512)
            .transpose(0, 2, 1, 3)).astype(bf),
        "b2": np.asarray(inputs["b2"], f32).copy(),
        "ln1_g": np.asarray(inputs["ln1_g"], f32).copy(),
        "ln1_b": np.asarray(inputs["ln1_b"], f32).copy(),
        "ln2_g": np.asarray(inputs["ln2_g"], f32).copy(),
        "ln2_b": np.asarray(inputs["ln2_b"], f32).copy(),
    }
    x = np.asarray(inputs["x"], f32)
    in_maps = [dict(common, x=np.ascontiguousarray(x[b])) for b in range(B)]
    return in_maps


def run_on_hw(inputs, trace=False):
    from concourse import bass_utils
    nc = _get_built()
    in_maps = prep_inputs(inputs)
    res = bass_utils.run_bass_kernel_spmd(nc, in_maps, core_ids=list(range(B)),
                                          trace=trace)
    out = np.stack([res.results[b]["out"] for b in range(B)], axis=0)
    return out, res


def _get_runner():
    """Cached sharded-jit runner so repeat kernel() calls skip recompilation."""
    if "runner" in _BUILT:
        return _BUILT["runner"]
    import jax
    from jax.sharding import Mesh, PartitionSpec, NamedSharding
    from jax.experimental.shard_map import shard_map
    from concourse import bass2jax
    import concourse.mybir as _mybir

    nc = _get_built()
    bass2jax.install_neuronx_cc_hook()
    partition_name = nc.partition_id_tensor.name if nc.partition_id_tensor else None
    in_names, out_names, out_avals = [], [], []
    for alloc in nc.m.functions[0].allocations:
        if not isinstance(alloc, _mybir.MemoryLocationSet):
            continue
        name = alloc.memorylocations[0].name
        if alloc.kind == "ExternalInput":
            if name != partition_name:
                in_names.append(name)
        elif alloc.kind == "ExternalOutput":
            out_names.append(name)
            out_avals.append(jax.core.ShapedArray(
                tuple(alloc.tensor_shape), _mybir.dt.np(alloc.dtype)))
    n_params = len(in_names)
    all_in = in_names + out_names + ([partition_name] if partition_name else [])

    def _body(*args):
        operands = list(args)
        if partition_name is not None:
            operands.append(bass2jax.partition_id_tensor())
        return tuple(bass2jax._bass_exec_p.bind(
            *operands, out_avals=tuple(out_avals), in_names=tuple(all_in),
            out_names=tuple(out_names), lowering_input_output_aliases=(),
            sim_require_finite=True, sim_require_nnan=True, nc=nc))

    devices = jax.devices()[:B]
    mesh = Mesh(np.array(devices), ("core",))
    n_outs = len(out_names)
    sharded = jax.jit(
        shard_map(_body, mesh=mesh,
                  in_specs=(PartitionSpec("core"),) * (n_params + n_outs),
                  out_specs=(PartitionSpec("core"),) * n_outs,
                  check_rep=False),
        keep_unused=True)
    zeros = [np.zeros((B * av.shape[0], *av.shape[1:]), av.dtype) for av in out_avals]

    def run(in_maps):
        concat = [np.concatenate([np.asarray(m[n]) for m in in_maps], axis=0)
                  for n in in_names]
        outs = sharded(*concat, *zeros)
        oi = out_names.index("out")
        full = np.asarray(outs[oi]).reshape(B, *out_avals[oi].shape)
        return full

    _BUILT["runner"] = run
    return run


def kernel(**inputs):
    in_maps = prep_inputs(inputs)
    try:
        run = _get_runner()
        return run(in_maps)
    except Exception:
        from concourse import bass_utils
        nc = _get_built()
        res = bass_utils.run_bass_kernel_spmd(nc, in_maps, core_ids=list(range(B)))
        return np.stack([res.results[b]["out"] for b in range(B)], axis=0)


def make_test_inputs(seed=0):
    rng = np.random.default_rng(seed)
    return {
        "x": rng.standard_normal((B, T, D)).astype(np.float32),
        "ln1_g": np.ones(D, np.float32), "ln1_b": np.zeros(D, np.float32),
        "ln2_g": np.ones(D, np.float32), "ln2_b": np.zeros(D, np.float32),
        "Wq": (rng.standard_normal((H, D, HD)) * 0.02).astype(np.float32),
        "bq": np.zeros((H, HD), np.float32),
        "Wk": (rng.standard_normal((H, D, HD)) * 0.02).astype(np.float32),
        "bk": np.zeros((H, HD), np.float32),
        "Wv": (rng.standard_normal((H, D, HD)) * 0.02).astype(np.float32),
        "bv": np.zeros((H, HD), np.float32),
        "Wo": (rng.standard_normal((D, D)) * 0.02).astype(np.float32),
        "bo": np.zeros(D, np.float32),
        "W1": (rng.standard_normal((D, FF)) * 0.02).astype(np.float32),
        "b1": np.zeros(FF, np.float32),
        "W2": (rng.standard_normal((FF, D)) * 0.02).astype(np.float32),
        "b2": np.zeros(D, np.float32),
    }


def np_ref_single(ins, xb, gelu="erf"):
    """float64 numpy reference for one batch element."""
    from scipy.special import erf

    def ln(v):
        mu = v.mean(-1, keepdims=True)
        var = ((v - mu) ** 2).mean(-1, keepdims=True)
        return (v - mu) / np.sqrt(var + 1e-5)

    Wq = np.transpose(ins["Wq"], (1, 0, 2)).reshape(D, D)
    Wk = np.transpose(ins["Wk"], (1, 0, 2)).reshape(D, D)
    Wv = np.transpose(ins["Wv"], (1, 0, 2)).reshape(D, D)
    h = ln(xb) * ins["ln1_g"] + ins["ln1_b"]
    q = h @ Wq + ins["bq"].reshape(-1)
    k = h @ Wk + ins["bk"].reshape(-1)
    v = h @ Wv + ins["bv"].reshape(-1)
    ctxs = []
    for hh in range(H):
        sl = slice(hh * HD, hh * HD + HD)
        sc = q[:, sl] @ k[:, sl].T / np.sqrt(HD)
        a = np.exp(sc - sc.max(-1, keepdims=True))
        a /= a.sum(-1, keepdims=True)
        ctxs.append(a @ v[:, sl])
    ctx = np.concatenate(ctxs, -1)
    xb = xb + ctx @ ins["Wo"] + ins["bo"]
    h2 = ln(xb) * ins["ln2_g"] + ins["ln2_b"]
    ff1 = h2 @ ins["W1"] + ins["b1"]
    if gelu == "tanh":
        g = np.tanh(ff1)
    else:
        g = 0.5 * ff1 * (1 + erf(ff1 / np.sqrt(2)))
    return xb + g @ ins["W2"] + ins["b2"]


if __name__ == "__main__":
    import sys
    mode = sys.argv[1] if len(sys.argv) > 1 else "sim"
    ins = make_test_inputs()
    if mode == "sim":
        import kernel as _self
        globals()["GELU_AF"] = AF.Tanh
        nc = bacc.Bacc("TRN2", target_bir_lowering=False, debug=False,
                       enable_asserts=False)
        build_block_kernel(nc)
        in_map = prep_inputs(ins)[0]
        from concourse.bass_interp import CoreSim
        sim = CoreSim(nc, trace=False)
        for name, arr in in_map.items():
            sim.tensor(name)[:] = arr
        sim.simulate()
        got = np.array(sim.tensor("out"))
        ref = np_ref_single(ins, ins["x"][0].astype(np.float64), gelu="tanh")
        rel = np.linalg.norm(got - ref) / np.linalg.norm(ref)
        print(f"sim maxabs={np.abs(got - ref).max():.5f} relnorm={rel:.6f}")
    else:
        out, res = run_on_hw(ins, trace=False)
        ref = np_ref_single(ins, ins["x"][0].astype(np.float64))
        rel = np.linalg.norm(out[0] - ref) / np.linalg.norm(ref)
        print(f"hw b0 maxabs={np.abs(out[0] - ref).max():.5f} relnorm={rel:.6f}")



# revision 41
# speedup vs baseline: 1.3533x; 1.3533x over previous
"""Trainium2 Bass kernel for nn_Block_13709535609166 (dense transformer block).

B=8, T=1024, D=1024, H=16, HD=64, FF=4096. Data-parallel over batch: one
batch element per NeuronCore (8 cores), no collectives. All matmuls bf16
with fp32 PSUM accumulation; residual stream and LN/softmax arithmetic fp32.

Self-contained: hardcodes shapes/sharding; only needs numpy/ml_dtypes and
the concourse (Bass) stack available in the container image.
"""
import numpy as np
import ml_dtypes

import concourse.bass as bass
import concourse.mybir as mybir
import concourse.tile as tile
from concourse import bacc
from concourse.masks import make_identity

BF16 = mybir.dt.bfloat16
F8 = mybir.dt.float8e4
F32 = mybir.dt.float32
AF = mybir.ActivationFunctionType
ALU = mybir.AluOpType
DR = mybir.MatmulPerfMode.DoubleRow
GELU_AF = AF.Gelu  # swapped to Tanh for CoreSim (Gelu not implemented in sim)

B, T, D, H = 8, 1024, 1024, 16
HD = D // H  # 64
FF = 4 * D
TC = T // 128   # 8 token chunks
DC = D // 128   # 8 feature chunks
DC2 = DC // 2   # 4 fp8 DoubleRow pair chunks
TC2 = TC // 2   # 4 token-chunk pairs (fp8 DR ctx contraction)
FC = FF // 128  # 32 ff chunks
NT = T // 512   # 2 free-dim chunks of 512 tokens
NF = D // 512   # 2 free-dim chunks of 512 features
W8SCALE = 64.0  # fp8 weights are stored x64 to stay out of subnormals


def build_block_kernel(nc, reps=1, loop_n=0):
    """Emit the full transformer block for one batch element.

    reps>1 re-applies the block on its own output (SBUF-chained); loop_n>0
    wraps the body in a hardware For loop -- both only for timing NEFFs so
    the (tens of ms) axon RPC overhead can be divided away.
    """
    dram = {}
    for name, shape, dt in [
        ("x", [T, D], F32),
        ("wq", [D, D], F8), ("wk", [D, D], F8), ("wv", [D, D], F8),
        ("bq", [D], F32), ("bk", [D], F32), ("bv", [D], F32),
        ("wo", [D, D], F8), ("bo", [D], F32), ("bo_bf", [D], BF16), ("b2_bf", [D], BF16), ("bv_bf", [D], BF16),
        ("w1", [FC, 128, DC, 128], BF16), ("b1", [FF], F32),
        ("w2", [FC, NF, 128, 512], BF16), ("b2", [D], F32),
        ("ln1_g", [D], F32), ("ln1_b", [D], F32),
        ("ln2_g", [D], F32), ("ln2_b", [D], F32),
    ]:
        dram[name] = nc.dram_tensor(name, shape, dt, kind="ExternalInput").ap()
    out_d = nc.dram_tensor("out", [T, D], F32, kind="ExternalOutput").ap()
    out_r = out_d.rearrange("(m p) d -> p m d", p=128)

    with tile.TileContext(nc) as tc:
        _emit(nc, tc, dram, out_r, reps, loop_n)
    return nc


def _emit(nc, tc, dram, out_r, reps=1, loop_n=0):
    from contextlib import ExitStack

    with ExitStack() as ctx:
        consts = ctx.enter_context(tc.tile_pool(name="consts", bufs=1))
        resid = ctx.enter_context(tc.tile_pool(name="resid", bufs=1))
        work = ctx.enter_context(tc.tile_pool(name="work", bufs=4))

        # ---- constants ----
        ident = consts.tile([128, 128], BF16)
        make_identity(nc, ident)
        ones65 = consts.tile([65, 64], BF16)
        nc.vector.memset(ones65, 1.0)
        eps_t = consts.tile([128, 1], F32)
        nc.vector.memset(eps_t, 1e-5)

        # ---- residual stream (token-major fp32, updated in place) ----
        x_sb = resid.tile([128, TC, D], F32)
        x_r = dram["x"].rearrange("(m p) d -> p m d", p=128)
        nc.sync.dma_start(x_sb[:, 0, :], x_r[:, 0, :])
        nc.sync.dma_start(x_sb[:, 1, :], x_r[:, 1, :])

        col = {}
        for name in ["bq", "bk"]:
            col[name] = consts.tile([128, DC], F32, name=f"c_{name}")
            nc.gpsimd.dma_start(col[name], dram[name].rearrange("(o p) -> p o", p=128))
        col["b1"] = consts.tile([128, FC], F32, name="c_b1")
        nc.gpsimd.dma_start(col["b1"], dram["b1"].rearrange("(o p) -> p o", p=128))
        # bv/bo/b2 are folded into the matmul accumulation via a ones-row matmul
        ones_row = consts.tile([1, 128], BF16, name="ones_row")
        nc.vector.memset(ones_row, 1.0)
        brow = {}
        for name in ["bv", "bo", "b2"]:
            brow[name] = consts.tile([1, D], BF16, name=f"br_{name}")
            nc.gpsimd.dma_start(brow[name], dram[name + "_bf"][None, :])

        hnorm_pool = ctx.enter_context(tc.tile_pool(name="hnorm", bufs=4))

        def ln_stats(x_src, m):
            """Token-major LN stats for chunk m -> normalized bf16 tile."""
            stats = work.tile([128, 2, 6], F32, name="stats")
            nc.vector.bn_stats(out=stats[:, 0, :], in_=x_src[:, m, 0:512])
            nc.vector.bn_stats(out=stats[:, 1, :], in_=x_src[:, m, 512:1024])
            mv = work.tile([128, 2], F32, name="mv")
            nc.vector.bn_aggr(out=mv, in_=stats)
            std = work.tile([128, 1], F32, name="std")
            nc.scalar.activation(out=std, in_=mv[:, 1:2], func=AF.Sqrt,
                                 bias=eps_t, scale=1.0)
            rstd = work.tile([128, 1], F32, name="rstd")
            nc.vector.reciprocal(rstd, std)
            h_norm = hnorm_pool.tile([128, D], BF16, name="h_norm")
            nc.any.tensor_scalar(
                out=h_norm, in0=x_src[:, m, :], scalar1=mv[:, 0:1], scalar2=rstd,
                op0=ALU.subtract, op1=ALU.mult)
            return h_norm

        def ln_transpose(h_norm, m, h_fm, tr_pool, tag="tr"):
            # LN gain/bias are folded into the weights/biases on the host, so
            # the transpose drain is a plain copy; nc.any lets the scheduler
            # put it on whichever of ACT/DVE is idle in this phase.
            for ko in range(DC):
                tr_ps = tr_pool.tile([128, 128], BF16, name=tag)
                nc.tensor.transpose(tr_ps, h_norm[:, 128 * ko:128 * ko + 128], ident)
                nc.any.tensor_copy(
                    out=h_fm[:, ko, 128 * m:128 * m + 128], in_=tr_ps)

        from contextlib import nullcontext
        loop_ctx = tc.For_i(0, loop_n, 1) if loop_n else nullcontext()
        with loop_ctx:
         for rep in range(reps):
            # Long-lived pools, opened early to respect LIFO pool discipline.
            # Stack (bottom->top): h2 | ctx | wo | qkv | <phase-local pools>
            st = ExitStack()
            p_h2 = st.enter_context(tc.tile_pool(name="h2_sb", bufs=1))
            h2_bf = p_h2.tile([128, DC, T], BF16, name="h2_bf")
            st_ctx = ExitStack()
            p_ctx = st_ctx.enter_context(tc.tile_pool(name="ctx_sb", bufs=1))
            ctx8 = p_ctx.tile([128, DC2, 2, T], F8, name="ctx8")
            st_wo = ExitStack()
            p_wo = st_wo.enter_context(tc.tile_pool(name="wo_sb", bufs=1))
            wo_sb = p_wo.tile([128, DC2, 2, D], F8, name="wo_sb")
            st_qkv = ExitStack()
            p_qkv = st_qkv.enter_context(tc.tile_pool(name="qkv_sb", bufs=1))
            q_fm = p_qkv.tile([128, DC, T], BF16, name="q_fm")
            k_fm = p_qkv.tile([128, DC, T], BF16, name="k_fm")
            # V with a ones column appended per head: [s, head, 0:64]=V, [.,.,64]=1
            # so the ctx matmul also produces the softmax denominator in row 64.
            # fp8, with source-token chunk PAIRS interleaved for DoubleRow ctx.
            v65 = p_qkv.tile([128, TC2, 2, H, 65], F8, name="v65")

            # ================= Phase 1: LN1 + QKV (fp8 DoubleRow) ==========
            # Weights live x64-scaled in fp8; h (LN1 out) is cast to fp8 in
            # the transpose-apply. ko chunk pairs are contracted 2-at-a-time
            # via perf_mode=DoubleRow (lhsT AP [128, 2, m], rhs [128, 2, n]).
            st_p1 = ExitStack()
            p1 = st_p1.enter_context(tc.tile_pool(name="p1_sb", bufs=1))
            tr_pool = st_p1.enter_context(tc.tile_pool(name="p1_tr", bufs=3, space="PSUM"))
            mm_pool = st_p1.enter_context(tc.tile_pool(name="p1_mm", bufs=5, space="PSUM"))

            h8 = p1.tile([128, DC2, 2, T], F8, name="h8")
            h8_flat = h8.rearrange("p a b t -> p (a b) t")
            h_norms = [None] * TC
            h_norms[0] = ln_stats(x_sb, 0)
            h_norms[1] = ln_stats(x_sb, 1)
            w8r = {n: dram[n].rearrange("(pr j p) q -> p pr j q", p=128, j=2)
                   for n in ("wq", "wk", "wv", "wo")}

            with tc.tile_pool(name="p1_w", bufs=3) as p_w:
                wv_sb = p_w.tile([128, DC2, 2, D], F8, name="w")
                wk_sb = p_w.tile([128, DC2, 2, D], F8, name="w")
                wq_sb = p_w.tile([128, DC2, 2, D], F8, name="w")
                for _pr in range(DC2):
                    for _j in range(2):
                        nc.sync.dma_start(wv_sb[:, _pr, _j, :], w8r["wv"][:, _pr, _j, :])
                    nc.sync.dma_start(x_sb[:, _pr + 2, :], x_r[:, _pr + 2, :])
                for _pr in range(DC2):
                    for _j in range(2):
                        nc.sync.dma_start(wk_sb[:, _pr, _j, :], w8r["wk"][:, _pr, _j, :])
                    if _pr < 2:
                        nc.sync.dma_start(x_sb[:, _pr + 6, :], x_r[:, _pr + 6, :])
                for _pr in range(DC2):
                    for _j in range(2):
                        nc.sync.dma_start(wq_sb[:, _pr, _j, :], w8r["wq"][:, _pr, _j, :])
                nc.vector.memset(v65[:, :, :, :, 64:65], 1.0)
                # V: token-major [t, vfeat]; lhsT = h8 pair chunk, rhs = W.
                # Transposes for chunk m emitted right before V[m] so PE never
                # waits in-order on DVE-produced h_norm.
                for m in range(TC):
                    ln_transpose(h_norms[m], m, h8_flat, tr_pool)
                    if m + 2 < TC:
                        h_norms[m + 2] = ln_stats(x_sb, m + 2)
                    ps = [mm_pool.tile([128, 512], F32, name="mm") for _ in range(NF)]
                    for pr in range(DC2):
                        for fn in range(NF):
                            nc.tensor.matmul(
                                ps[fn], lhsT=h8[:, pr, :, 128 * m:128 * m + 128],
                                rhs=wv_sb[:, pr, :, 512 * fn:512 * fn + 512],
                                start=(pr == 0), stop=False, perf_mode=DR)
                    for fn in range(NF):
                        nc.tensor.matmul(ps[fn], lhsT=ones_row,
                                         rhs=brow["bv"][:, 512 * fn:512 * fn + 512],
                                         start=False, stop=True)
                        nc.scalar.activation(
                            out=v65[:, m // 2, m % 2, 8 * fn:8 * fn + 8, 0:64],
                            in_=ps[fn].rearrange("p (h e) -> p h e", e=64),
                            func=AF.Copy, scale=1.0 / W8SCALE)
                # K, Q interleaved per m so attention pair m can start early
                for m in range(DC):
                    for dst, wt, bias in [(k_fm, wk_sb, col["bk"]), (q_fm, wq_sb, col["bq"])]:
                        ps = [mm_pool.tile([128, 512], F32, name="mm") for _ in range(NT)]
                        for pr in range(DC2):
                            for tn in range(NT):
                                nc.tensor.matmul(
                                    ps[tn], lhsT=wt[:, pr, :, 128 * m:128 * m + 128],
                                    rhs=h8[:, pr, :, 512 * tn:512 * tn + 512],
                                    start=(pr == 0), stop=(pr == DC2 - 1), perf_mode=DR)
                        for tn in range(NT):
                            nc.any.tensor_scalar(
                                out=dst[:, m, 512 * tn:512 * tn + 512], in0=ps[tn],
                                scalar1=1.0 / W8SCALE, scalar2=bias[:, m:m + 1],
                                op0=ALU.mult, op1=ALU.add)
            st_p1.close()

            # ========== Phase 2+3: attention / out-proj pipelined ==========
            for _pr in range(DC2):
                for _j in range(2):
                    nc.sync.dma_start(wo_sb[:, _pr, _j, :], w8r["wo"][:, _pr, _j, :])

            h2_norms = [None] * TC
            MH = TC // 2  # token chunks per tn half

            def proj_m(m, mm3_pool):
                """Out-proj + residual + LN2 + h2 transpose for token chunk m."""
                for fn in range(NF):
                    sl = slice(512 * fn, 512 * fn + 512)
                    ps = mm3_pool.tile([128, 512], F32, name="pr")
                    for pr in range(DC2):
                        nc.tensor.matmul(
                            ps, lhsT=ctx8[:, pr, :, 128 * m:128 * m + 128],
                            rhs=wo_sb[:, pr, :, sl],
                            start=(pr == 0), stop=False, perf_mode=DR)
                    nc.tensor.matmul(ps, lhsT=ones_row, rhs=brow["bo"][:, sl],
                                     start=False, stop=True)
                    # x += attn_out/W8SCALE  (bias row is pre-scaled on host)
                    nc.vector.scalar_tensor_tensor(
                        out=x_sb[:, m, sl], in0=ps, scalar=1.0 / W8SCALE,
                        in1=x_sb[:, m, sl], op0=ALU.mult, op1=ALU.add)
                h2_norms[m] = ln_stats(x_sb, m)
                ln_transpose(h2_norms[m], m, h2_bf, mm3_pool, tag="pr")

            # PSUM stack: pr (bottom, 2 banks) | sc (4 banks) | cx (2 banks)
            st_pr = ExitStack()
            pr_pool = st_pr.enter_context(tc.tile_pool(name="p3_pr", bufs=2, space="PSUM"))
            st_att = ExitStack()
            e_pool = st_att.enter_context(tc.tile_pool(name="p2_e", bufs=6))
            sc_pool = st_att.enter_context(tc.tile_pool(name="p2_sc", bufs=2, space="PSUM"))
            cx_pool = st_att.enter_context(tc.tile_pool(name="p2_cx", bufs=1, space="PSUM"))
            # Flat software pipeline over (tn, pc, sm): the two heads' score
            # MMs are adjacent (disjoint PE row groups 0-63/64-127 -> they
            # run concurrently in the array on HW); ctx MMs trail the exp by
            # DEPTH steps; each head-pair's normalize chain is deferred into
            # the next pair's stream so neither PE nor ACT ever waits on it.
            DEPTH = 2
            steps = [(tn, pc, b)
                     for tn in range(NT) for pc in range(DC) for b in range(TC2)]
            cx_of = {}   # (tn, pc) -> [cx_ps_h0, cx_ps_h1]
            e_of = {}    # step idx -> [e_h0, e_h1]

            def ctx_mms(idx):
                tn, pc, b = steps[idx]
                if b == 0:
                    cx_of[(tn, pc)] = [cx_pool.tile([65, 512], F32, name=f"cx{h}")
                                       for h in range(2)]
                es = e_of.pop(idx)
                for h in range(2):
                    # fp8 DoubleRow: contracts the sm-chunk PAIR (256 virtual
                    # rows) in one 512-cycle pass.
                    nc.tensor.matmul(
                        cx_of[(tn, pc)][h],
                        lhsT=v65[:, b, :, 2 * pc + h, :], rhs=es[h],
                        start=(b == 0), stop=(b == TC2 - 1), perf_mode=DR)

            def normalize(tn, pc):
                tsl = slice(512 * tn, 512 * tn + 512)
                cx_ps = cx_of.pop((tn, pc))
                for h in range(2):
                    lo = 64 * h
                    rec = work.tile([65, 512], BF16, name="rec")
                    with nc.allow_low_precision("softmax denom recip in bf16"):
                        nc.vector.reciprocal(rec[64:65, :], cx_ps[h][64:65, :])
                    rb_ps = pr_pool.tile([128, 512], F32, name="pr")
                    nc.tensor.matmul(
                        rb_ps[0:64, :], lhsT=ones65[64:65, :], rhs=rec[64:65, :],
                        start=True, stop=True)
                    # walrus forbids two PSUM reads in one DVE op -> copy the
                    # unnormalized ctx (can be ~1e3, too big for fp8) to a
                    # bf16 staging tile, then scale into the fp8 ctx store.
                    tmp = work.tile([64, 512], BF16, name="cxc")
                    nc.vector.tensor_copy(out=tmp, in_=cx_ps[h][0:64, :])
                    nc.vector.tensor_mul(out=ctx8[lo:lo + 64, pc // 2, pc % 2, tsl],
                                         in0=tmp, in1=rb_ps[0:64, :])

            for idx, (tn, pc, b) in enumerate(steps):
                tsl = slice(512 * tn, 512 * tn + 512)
                es = []
                for h in range(2):
                    lo = 64 * h
                    sc = sc_pool.tile([128, 2, 512], F32, name="sc")
                    for i in range(2):
                        sm = 2 * b + i
                        nc.tensor.matmul(
                            sc[:, i, :],
                            lhsT=k_fm[lo:lo + 64, pc, 128 * sm:128 * sm + 128],
                            rhs=q_fm[lo:lo + 64, pc, tsl],
                            start=True, stop=True)
                    e = e_pool.tile([128, 2, 512], F8, name="e")
                    nc.scalar.activation(out=e, in_=sc, func=AF.Exp, scale=0.125)
                    es.append(e)
                e_of[idx] = es
                if idx >= DEPTH:
                    ctx_mms(idx - DEPTH)
                if b == DEPTH - 1 and idx >= TC2:
                    ptn, ppc, _ = steps[idx - TC2]
                    normalize(ptn, ppc)
                # proj of the first token half interleaves into the second
                # attention half's emission so pr-slot allocation follows
                # usage order (no blocking behind later rb tiles).
                if tn == 1 and b == TC2 - 1 and pc % 2 == 1:
                    proj_m((pc - 1) // 2, pr_pool)
            for idx in range(len(steps) - DEPTH, len(steps)):
                ctx_mms(idx)
            normalize(*steps[-1][:2])
            for m in range(MH, TC):
                proj_m(m, pr_pool)
            st_att.close()
            st_qkv.close()

            # ================= Phase 4: FFN =================
            st_g1 = ExitStack()
            p_g1 = st_g1.enter_context(tc.tile_pool(name="g1_sb", bufs=1))
            g1_fm = p_g1.tile([128, FC, T], BF16, name="g1_fm")

            st_mm4 = ExitStack()
            mm4_pool = st_mm4.enter_context(
                tc.tile_pool(name="p4_mm1", bufs=4, space="PSUM"))
            st_w1 = ExitStack()
            w1_pool = st_w1.enter_context(tc.tile_pool(name="p4_w1", bufs=6))
            # mm1 in two passes (tn halves) so the tn=0 pass overlaps the tail
            # of attention/proj; W1 is re-streamed for the second pass.
            for tn in range(NT):
                tsl = slice(512 * tn, 512 * tn + 512)
                for mf in range(FC):
                    w1_t = w1_pool.tile([128, DC, 128], BF16, name="w1t")
                    nc.sync.dma_start(w1_t, dram["w1"][mf])
                    ps = mm4_pool.tile([128, 512], F32, name="mm")
                    for ko in range(DC):
                        nc.tensor.matmul(
                            ps, lhsT=w1_t[:, ko, :], rhs=h2_bf[:, ko, tsl],
                            start=(ko == 0), stop=(ko == DC - 1))
                    nc.scalar.activation(
                        out=g1_fm[:, mf, tsl], in_=ps,
                        func=GELU_AF, bias=col["b1"][:, mf:mf + 1], scale=1.0)
            st_w1.close()
            st_mm4.close()
            st_pr.close()

            st_mm4b = ExitStack()
            mm4b_pool = st_mm4b.enter_context(
                tc.tile_pool(name="p4_mm2", bufs=8, space="PSUM"))
            st_w2 = ExitStack()
            w2_pool = st_w2.enter_context(tc.tile_pool(name="p4_w2", bufs=4))
            with (
                tc.tile_pool(name="p4_out", bufs=4) as out_pool,
            ):
                for th in range(2):
                    ps = [[mm4b_pool.tile([128, 512], F32, name="mm") for _ in range(NF)]
                          for _ in range(MH)]
                    for ko in range(FC):
                        w2_t = w2_pool.tile([128, NF, 512], BF16, name="w2t")
                        nc.sync.dma_start(w2_t, dram["w2"][ko].rearrange("f p c -> p f c"))
                        for m4 in range(MH):
                            m = MH * th + m4
                            for fn in range(NF):
                                nc.tensor.matmul(
                                    ps[m4][fn], lhsT=g1_fm[:, ko, 128 * m:128 * m + 128],
                                    rhs=w2_t[:, fn, :],
                                    start=(ko == 0), stop=False)
                    for m4 in range(MH):
                        m = MH * th + m4
                        for fn in range(NF):
                            sl = slice(512 * fn, 512 * fn + 512)
                            nc.tensor.matmul(ps[m4][fn], lhsT=ones_row,
                                             rhs=brow["b2"][:, sl], start=False, stop=True)
                            o = out_pool.tile([128, 512], F32, name="o")
                            nc.vector.tensor_add(out=o, in0=ps[m4][fn], in1=x_sb[:, m, sl])
                            if rep < reps - 1 or loop_n:
                                nc.vector.tensor_scalar(
                                    out=x_sb[:, m, sl], in0=o, scalar1=0.5,
                                    scalar2=None, op0=ALU.mult)
                            if rep == reps - 1:
                                nc.sync.dma_start(out_r[:, m, sl], o)
            st_w2.close()
            st_mm4b.close()
            st_g1.close()
            st_wo.close()
            st_ctx.close()
            st.close()


_BUILT = {}


def _get_built():
    if "nc" not in _BUILT:
        nc = bacc.Bacc("TRN2", target_bir_lowering=False, debug=False,
                       enable_asserts=False, num_devices=8)
        build_block_kernel(nc)
        nc.compile()
        _BUILT["nc"] = nc
    return _BUILT["nc"]


def prep_inputs(inputs):
    """Host-side reshape/cast of the full (unsharded) inputs.

    LayerNorm affine params are folded in here: gains scale the weight rows
    (h_norm @ diag(g) @ W == h_norm @ (g[:,None]*W)) and biases fold into the
    downstream bias vectors (b_ln @ W + b).
    """
    bf = ml_dtypes.bfloat16
    f8 = ml_dtypes.float8_e4m3
    f32 = np.float32

    g1 = np.asarray(inputs["ln1_g"], f32)
    b1ln = np.asarray(inputs["ln1_b"], f32)
    g2 = np.asarray(inputs["ln2_g"], f32)
    b2ln = np.asarray(inputs["ln2_b"], f32)

    def flat_heads(w):  # [H, D, HD] -> [D, H*HD] f32
        return np.ascontiguousarray(np.transpose(np.asarray(w, f32), (1, 0, 2))
                                    .reshape(D, D))

    Wq_f, Wk_f, Wv_f = (flat_heads(inputs[n]) for n in ("Wq", "Wk", "Wv"))
    bq_e = np.asarray(inputs["bq"], f32).reshape(D) + b1ln @ Wq_f
    bk_e = np.asarray(inputs["bk"], f32).reshape(D) + b1ln @ Wk_f
    bv_e = np.asarray(inputs["bv"], f32).reshape(D) + b1ln @ Wv_f
    W1_f = np.asarray(inputs["W1"], f32)
    b1_e = np.asarray(inputs["b1"], f32) + b2ln @ W1_f

    common = {
        "wq": (g1[:, None] * Wq_f * W8SCALE).astype(f8),
        "wk": (g1[:, None] * Wk_f * W8SCALE).astype(f8),
        "wv": (g1[:, None] * Wv_f * W8SCALE).astype(f8),
        "bq": bq_e, "bk": bk_e, "bv": bv_e,
        "wo": (np.ascontiguousarray(np.asarray(inputs["Wo"], f32)) * W8SCALE).astype(f8),
        "bo": np.asarray(inputs["bo"], f32).copy(),
        "bo_bf": (np.asarray(inputs["bo"], f32) * W8SCALE).astype(bf),
        "b2_bf": np.asarray(inputs["b2"], f32).astype(bf),
        "bv_bf": (bv_e * W8SCALE).astype(bf),
        "w1": np.ascontiguousarray(
            (g2[:, None] * W1_f).reshape(DC, 128, FC, 128)
            .transpose(2, 1, 0, 3)).astype(bf),
        "b1": b1_e,
        "w2": np.ascontiguousarray(
            np.asarray(inputs["W2"], f32).reshape(FC, 128, NF, 512)
            .transpose(0, 2, 1, 3)).astype(bf),
        "b2": np.asarray(inputs["b2"], f32).copy(),
        "ln1_g": np.asarray(inputs["ln1_g"], f32).copy(),
        "ln1_b": np.asarray(inputs["ln1_b"], f32).copy(),
        "ln2_g": np.asarray(inputs["ln2_g"], f32).copy(),
        "ln2_b": np.asarray(inputs["ln2_b"], f32).copy(),
    }
    x = np.asarray(inputs["x"], f32)
    in_maps = [dict(common, x=np.ascontiguousarray(x[b])) for b in range(B)]
    return in_maps


def run_on_hw(inputs, trace=False):
    from concourse import bass_utils
    nc = _get_built()
    in_maps = prep_inputs(inputs)
    res = bass_utils.run_bass_kernel_spmd(nc, in_maps, core_ids=list(range(B)),
                                          trace=trace)
    out = np.stack([res.results[b]["out"] for b in range(B)], axis=0)
    return out, res


def _get_runner():
    """Cached sharded-jit runner so repeat kernel() calls skip recompilation."""
    if "runner" in _BUILT:
        return _BUILT["runner"]
    import jax
    from jax.sharding import Mesh, PartitionSpec, NamedSharding
    from jax.experimental.shard_map import shard_map
    from concourse import bass2jax
    import concourse.mybir as _mybir

    nc = _get_built()
    bass2jax.install_neuronx_cc_hook()
    partition_name = nc.partition_id_tensor.name if nc.partition_id_tensor else None
    in_names, out_names, out_avals = [], [], []
    for alloc in nc.m.functions[0].allocations:
        if not isinstance(alloc, _mybir.MemoryLocationSet):
            continue
        name = alloc.memorylocations[0].name
        if alloc.kind == "ExternalInput":
            if name != partition_name:
                in_names.append(name)
        elif alloc.kind == "ExternalOutput":
            out_names.append(name)
            out_avals.append(jax.core.ShapedArray(
                tuple(alloc.tensor_shape), _mybir.dt.np(alloc.dtype)))
    n_params = len(in_names)
    all_in = in_names + out_names + ([partition_name] if partition_name else [])

    def _body(*args):
        operands = list(args)
        if partition_name is not None:
            operands.append(bass2jax.partition_id_tensor())
        return tuple(bass2jax._bass_exec_p.bind(
            *operands, out_avals=tuple(out_avals), in_names=tuple(all_in),
            out_names=tuple(out_names), lowering_input_output_aliases=(),
            sim_require_finite=True, sim_require_nnan=True, nc=nc))

    devices = jax.devices()[:B]
    mesh = Mesh(np.array(devices), ("core",))
    n_outs = len(out_names)
    sharded = jax.jit(
        shard_map(_body, mesh=mesh,
                  in_specs=(PartitionSpec("core"),) * (n_params + n_outs),
                  out_specs=(PartitionSpec("core"),) * n_outs,
                  check_rep=False),
        keep_unused=True)
    zeros = [np.zeros((B * av.shape[0], *av.shape[1:]), av.dtype) for av in out_avals]

    def run(in_maps):
        concat = [np.concatenate([np.asarray(m[n]) for m in in_maps], axis=0)
                  for n in in_names]
        outs = sharded(*concat, *zeros)
        oi = out_names.index("out")
        full = np.asarray(outs[oi]).reshape(B, *out_avals[oi].shape)
        return full

    _BUILT["runner"] = run
    return run


def kernel(**inputs):
    in_maps = prep_inputs(inputs)
    try:
        run = _get_runner()
        return run(in_maps)
    except Exception:
        from concourse import bass_utils
        nc = _get_built()
        res = bass_utils.run_bass_kernel_spmd(nc, in_maps, core_ids=list(range(B)))
        return np.stack([res.results[b]["out"] for b in range(B)], axis=0)


def make_test_inputs(seed=0):
    rng = np.random.default_rng(seed)
    return {
        "x": rng.standard_normal((B, T, D)).astype(np.float32),
        "ln1_g": np.ones(D, np.float32), "ln1_b": np.zeros(D, np.float32),
        "ln2_g": np.ones(D, np.float32), "ln2_b": np.zeros(D, np.float32),
        "Wq": (rng.standard_normal((H, D, HD)) * 0.02).astype(np.float32),
        "bq": np.zeros((H, HD), np.float32),
        "Wk": (rng.standard_normal((H, D, HD)) * 0.02).astype(np.float32),
        "bk": np.zeros((H, HD), np.float32),
        "Wv": (rng.standard_normal((H, D, HD)) * 0.02).astype(np.float32),
        "bv": np.zeros((H, HD), np.float32),
        "Wo": (rng.standard_normal((D, D)) * 0.02).astype(np.float32),
        "bo": np.zeros(D, np.float32),
        "W1": (rng.standard_normal((D, FF)) * 0.02).astype(np.float32),
        "b1": np.zeros(FF, np.float32),
        "W2": (rng.standard_normal((FF, D)) * 0.02).astype(np.float32),
        "b2": np.zeros(D, np.float32),
    }


def np_ref_single(ins, xb, gelu="erf"):
    """float64 numpy reference for one batch element."""
    from scipy.special import erf

    def ln(v):
        mu = v.mean(-1, keepdims=True)
        var = ((v - mu) ** 2).mean(-1, keepdims=True)
        return (v - mu) / np.sqrt(var + 1e-5)

    Wq = np.transpose(ins["Wq"], (1, 0, 2)).reshape(D, D)
    Wk = np.transpose(ins["Wk"], (1, 0, 2)).reshape(D, D)
    Wv = np.transpose(ins["Wv"], (1, 0, 2)).reshape(D, D)
    h = ln(xb) * ins["ln1_g"] + ins["ln1_b"]
    q = h @ Wq + ins["bq"].reshape(-1)
    k = h @ Wk + ins["bk"].reshape(-1)
    v = h @ Wv + ins["bv"].reshape(-1)
    ctxs = []
    for hh in range(H):
        sl = slice(hh * HD, hh * HD + HD)
        sc = q[:, sl] @ k[:, sl].T / np.sqrt(HD)
        a = np.exp(sc - sc.max(-1, keepdims=True))
        a /= a.sum(-1, keepdims=True)
        ctxs.append(a @ v[:, sl])
    ctx = np.concatenate(ctxs, -1)
    xb = xb + ctx @ ins["Wo"] + ins["bo"]
    h2 = ln(xb) * ins["ln2_g"] + ins["ln2_b"]
    ff1 = h2 @ ins["W1"] + ins["b1"]
    if gelu == "tanh":
        g = np.tanh(ff1)
    else:
        g = 0.5 * ff1 * (1 + erf(ff1 / np.sqrt(2)))
    return xb + g @ ins["W2"] + ins["b2"]


if __name__ == "__main__":
    import sys
    mode = sys.argv[1] if len(sys.argv) > 1 else "sim"
    ins = make_test_inputs()
    if mode == "sim":
        import kernel as _self
        globals()["GELU_AF"] = AF.Tanh
        nc = bacc.Bacc("TRN2", target_bir_lowering=False, debug=False,
                       enable_asserts=False)
        build_block_kernel(nc)
        in_map = prep_inputs(ins)[0]
        from concourse.bass_interp import CoreSim
        sim = CoreSim(nc, trace=False)
        for name, arr in in_map.items():
            sim.tensor(name)[:] = arr
        sim.simulate()
        got = np.array(sim.tensor("out"))
        ref = np_ref_single(ins, ins["x"][0].astype(np.float64), gelu="tanh")
        rel = np.linalg.norm(got - ref) / np.linalg.norm(ref)
        print(f"sim maxabs={np.abs(got - ref).max():.5f} relnorm={rel:.6f}")
    else:
        out, res = run_on_hw(ins, trace=False)
        ref = np_ref_single(ins, ins["x"][0].astype(np.float64))
        rel = np.linalg.norm(out[0] - ref) / np.linalg.norm(ref)
        print(f"hw b0 maxabs={np.abs(out[0] - ref).max():.5f} relnorm={rel:.6f}")

